# revision 1
# baseline (speedup 1.0000x reference)
"""ARD-RBF covariance kernel for Trainium2 (Bass/Tile), 8-core row-parallel.

Math (matches the reference):
    s  = exp(-weights[:, 0])                      # (D,) inverse lengthscales
    sq[i, j] = ||Us_i||^2 + ||Vs_j||^2 - 2 Us_i . Vs_j
    K[i, j]  = exp(2*sn) * exp(-0.5 * max(sq, 0))

Host side (inside kernel(), O(N*D) prep — 1e-4 of the total FLOPs):
    A  = -2 * s^2 * U_shard^T   split into bf16 hi+lo   # (16, 1024) per core
    B  = V^T                    split into bf16 hi+lo   # (16, 8192) replicated
    v2 = ||Vs||^2               split into bf16 a+b, riding two ones rows
    bias[p, m] = 2*sn - 0.5 * ||Us_{128m+p}||^2         # (128, 8) per core, f32

Device side (per core, rows sharded 8 ways):
    psum = [Ahi;1;1;Ahi;Alo]^T @ [Bhi;v2a;v2b;Blo;Bhi]
    (ONE bf16 matmul per 512-chunk, K=50: hi*hi + v2 + the two lo cross
    terms in a single pass; bf16 products are exact in the f32 PSUM,
    giving ~17 effective mantissa bits — measured 1.1e-3 rel err).  So
    psum = -2 Us.Vs + v2_j (minus the negligible Alo.Blo term).  One
    ScalarE activation per 2048-wide quad computes
    out = Exp(-0.5*psum + bias_m) (PSUM -> SBUF), DMA writes each 1 MB
    quad out.

    Operands carry 2 copies at partitions 0/64 so chunk matmuls alternate
    between two PE row groups (tile_position row tiling).  The preamble is
    pure DMA (~1.9 MB in).

The (8192, 8192) f32 output (256 MB) makes this memory-bound on the
HBM write (~93 us/core at ~358 GB/s); PE/ACT work is overlapped.
"""

import numpy as np

import concourse.bacc as bacc
import concourse.bass as bass  # noqa: F401  (AP helpers)
import concourse.mybir as mybir
import concourse.tile as tile

N, M, D = 8192, 8192, 16
N_CORES = 8
ROWS = N // N_CORES  # 1024 rows of U per core
P = 128              # output partitions per row block
FREE = 512           # matmul moving free dim (one PSUM bank of f32)
QUAD = 2048          # ACT chunk: 4 banks
KA = 3 * D + 2       # one stacked contraction: [Ahi;1;1;Ahi;Alo] x [Bhi;v2a;v2b;Blo;Bhi]

F32 = mybir.dt.float32
BF16 = mybir.dt.bfloat16
AF = mybir.ActivationFunctionType
NP_BF16 = mybir.dt.np(BF16)


def build_program(rows=ROWS, m_cols=M, repeats=1):
    """Build the per-core Bass program. rows/m_cols shrinkable for sim."""
    rb = rows // P
    nq = m_cols // QUAD

    nc = bacc.Bacc()
    lta = nc.declare_dram_parameter("lta", [KA, rows], BF16, isOutput=False)
    rta = nc.declare_dram_parameter("rta", [KA, m_cols], BF16, isOutput=False)
    bt = nc.declare_dram_parameter("bt", [P, rb], F32, isOutput=False)
    out = nc.declare_dram_parameter("out", [rows, m_cols], F32, isOutput=True)

    with tile.TileContext(nc) as tc:
        with (
            tc.tile_pool(name="singles", bufs=1) as singles,
            tc.tile_pool(name="psum_pool", bufs=2, space="PSUM") as psum_pool,
            tc.tile_pool(name="obuf_pool", bufs=4) as obuf_pool,
        ):
            # --- preamble: pure DMA ------------------------------------
            # Operands carry 2 copies at partitions 0/64 for the 2-way
            # tile_position row tiling.
            biasT = singles.tile([P, rb], F32)
            nc.sync.dma_start(biasT[:], bt[:])
            # Each dma_start costs ~650ns of HWDGE issue time, so the
            # preamble uses as few DMAs as possible: 5 total.
            LT = singles.tile([64 + KA, rows], BF16)
            RT = singles.tile([64 + KA, m_cols], BF16)
            for g in range(2):
                o = 64 * g
                nc.sync.dma_start(LT[o : o + KA, :], lta[:])
                nc.sync.dma_start(RT[o : o + KA, :], rta[:])

            # --- main loop ----------------------------------------------
            for _rep in range(repeats):
                for m in range(rb):
                    msl = slice(m * P, (m + 1) * P)
                    for q in range(nq):
                        first = m == 0 and q == 0
                        ps = psum_pool.tile([P, QUAD], F32, tag="ps", name="ps")
                        for k in range(QUAD // FREE):
                            n = q * (QUAD // FREE) + k
                            # the first quad runs entirely in row group 0 so
                            # it only waits for the first RT copy's DMA
                            o = 0 if first else 64 * (k % 2)
                            csl = slice(k * FREE, (k + 1) * FREE)
                            nsl = slice(n * FREE, (n + 1) * FREE)
                            nc.tensor.matmul(
                                ps[:, csl],
                                LT[o : o + KA, msl], RT[o : o + KA, nsl],
                                start=True, stop=True,
                                tile_position=(o, 0),
                            )
                        # the first quad goes out in 1024-wide halves so the
                        # store stream starts before the preamble finishes;
                        # steady state uses one 1MB store per quad
                        nh = 2 if first else 1
                        w = QUAD // nh
                        for h in range(nh):
                            ob = obuf_pool.tile([P, w], F32, tag="ob", name="ob")
                            nc.scalar.activation(
                                ob[:], ps[:, h * w : (h + 1) * w],
                                AF.Exp, bias=biasT[:, m : m + 1], scale=-0.5,
                            )
                            nc.sync.dma_start(
                                out[msl, q * QUAD + h * w : q * QUAD + (h + 1) * w],
                                ob[:],
                            )

    nc.compile()  # bacc lowering: splits multi-waits, reg alloc, etc.
    return nc


_PROGRAM_CACHE = {}


def get_program(rows=ROWS, m_cols=M, repeats=1):
    key = (rows, m_cols, repeats)
    if key not in _PROGRAM_CACHE:
        _PROGRAM_CACHE[key] = build_program(rows, m_cols, repeats)
    return _PROGRAM_CACHE[key]


def make_in_maps(U, V, weights, sn):
    U = np.asarray(U, dtype=np.float32)
    V = np.asarray(V, dtype=np.float32)
    w = np.asarray(weights, dtype=np.float32).reshape(D)
    sn_f = np.float64(np.asarray(sn, dtype=np.float32))

    s = np.exp(-w.astype(np.float64))
    s2 = s * s

    # R side: raw V^T split hi+lo, plus v2 = ||Vs||^2 split into two bf16
    # rows (a + b) that ride the two ones rows of the hi L operand.
    v2 = ((V.astype(np.float64) * s) ** 2).sum(axis=1)          # (M,)
    v2a = v2.astype(np.float32).astype(NP_BF16)
    v2b = (v2 - v2a.astype(np.float64)).astype(np.float32).astype(NP_BF16)
    Vt = np.ascontiguousarray(V.T)                              # (D, M) f32
    Vhi = Vt.astype(NP_BF16)
    Vlo = (Vt - Vhi.astype(np.float32)).astype(NP_BF16)
    rta = np.empty((KA, M), dtype=NP_BF16)
    rta[:D] = Vhi
    rta[D] = v2a
    rta[D + 1] = v2b
    rta[D + 2 : 2 * D + 2] = Vlo
    rta[2 * D + 2 :] = Vhi
    rta = np.ascontiguousarray(rta)

    in_maps = []
    for c in range(N_CORES):
        Uc = U[c * ROWS : (c + 1) * ROWS].astype(np.float64)    # (ROWS, D)
        A = ((Uc * (-2.0 * s2)).T).astype(np.float32)           # (D, ROWS)
        Ahi = A.astype(NP_BF16)
        Alo = (A - Ahi.astype(np.float32)).astype(NP_BF16)
        lta = np.empty((KA, ROWS), dtype=NP_BF16)
        lta[:D] = Ahi
        lta[D] = 1.0
        lta[D + 1] = 1.0
        lta[D + 2 : 2 * D + 2] = Ahi
        lta[2 * D + 2 :] = Alo
        u2 = ((Uc * s) ** 2).sum(axis=1)                        # (ROWS,)
        bias = (2.0 * sn_f - 0.5 * u2).reshape(ROWS // P, P).T  # (P, rb)
        in_maps.append({
            "lta": np.ascontiguousarray(lta),
            "rta": rta,
            "bt": np.ascontiguousarray(bias.astype(np.float32)),
        })
    return in_maps


def kernel(U, V, weights, sn):
    from concourse.bass_utils import run_bass_kernel_spmd

    nc = get_program()
    in_maps = make_in_maps(U, V, weights, sn)
    res = run_bass_kernel_spmd(nc, in_maps, core_ids=list(range(N_CORES)))
    return np.concatenate([r["out"] for r in res.results], axis=0)



# revision 2
# speedup vs baseline: 1.4991x; 1.4991x over previous
"""ARD-RBF covariance kernel for Trainium2 (Bass/Tile), 8-core row-parallel.

Math (matches the reference):
    s  = exp(-weights[:, 0])                      # (D,) inverse lengthscales
    sq[i, j] = ||Us_i||^2 + ||Vs_j||^2 - 2 Us_i . Vs_j
    K[i, j]  = exp(2*sn) * exp(-0.5 * max(sq, 0))

Strategy: f16 output (halves the HBM store vs f32) and the elementwise
exp split across BOTH ScalarE and VectorE so neither engine binds:

  The matmul produces, directly in PSUM,
      T[i,j] = (log2(K[i,j]) + 14.5) * 2^10
  via bf16 hi/lo-split operands (dot term) plus bias rows (row/col
  norms, 3-way bf16 splits riding ones-rows); K = 54 stacked rows, one
  bf16 matmul pass per 512-column chunk.

  - ScalarE quads: out_f16 = Exp(T * ln2/2^10 - 14.5*ln2)   (1 elem/cyc)
  - VectorE quads: custom DVE op EXPF16_ANT builds the f16 BIT PATTERN
    of 2^y in one fused pass (8 fp32 ALU stages): round T to a multiple
    of 2^10 (+-1.5*2^33 trick) -> exponent field q*2^10, remainder
    v in [-.5,.5], quadratic minimax p(v) ~= 2^(v+1/2)-1 for the
    mantissa (linear coeff pinned to the hw constant One), output
    (q + p(v))*2^10 with saturating-at-0 uint16 convert, which handles
    deep-negative exponents (underflow -> 0) and f16 subnormals
    gracefully (the int arithmetic carries/borrows across the exponent
    field boundary). One DVE instruction per 128x2048 quad at 1
    elem/cyc; measured ~1.8e-3 max rel err.

  Per-core exp work (8.4M elems) runs on two engines in parallel
  (~2.0us ScalarE + ~2.3us VectorE per quad), leaving the kernel bound
  by the f16 HBM store (16 MB/core).

Host side: O(N*D) prep + final uint16 -> float16 view -> float32 cast.
"""

import numpy as np

import concourse.bacc as bacc
import concourse.bass as bass  # noqa: F401
import concourse.mybir as mybir
import concourse.tile as tile

N, M, D = 8192, 8192, 16
N_CORES = 8
ROWS = N // N_CORES      # 1024 rows of U per core
P = 128                  # output partitions per row block
FREE = 512               # matmul moving free dim (one PSUM bank of f32)
QUAD = 2048              # one engine chunk: 4 PSUM banks
KA = 3 * D + 6           # stacked contraction rows (see make_in_maps)

F32 = mybir.dt.float32
F16 = mybir.dt.float16
U16 = mybir.dt.uint16
BF16 = mybir.dt.bfloat16
AF = mybir.ActivationFunctionType
NP_BF16 = mybir.dt.np(BF16)

LOG2E = 1.4426950408889634
LN2 = 0.6931471805599453
ALPHA = LOG2E * 2.0**10          # T = ALPHA*(2sn - 0.5*sq) + 14.5*2^10
EXP_BIAS = 14.5 * 2.0**10

# engine assignment pattern tiled over quads: A=ScalarE exp, D=VectorE op.
# Measured per-quad: ScalarE ~1.15us (f16-out Exp runs 2x), VectorE ~2.26us,
# so a 2:1 split balances the engines (~25us) under the ~31us DMA bound.
ASSIGN_PATTERN = "AAD"
# store granularity in columns (8192 = one 2 MB f16 store per 128-row block)
STORE_COLS = 8192

# --- EXPF16_ANT custom DVE op ------------------------------------------
# quadratic weighted-minimax fit of 2^(v+1/2)-1 on [-.5,.5] with the
# linear coefficient pinned to 1.0 (the DVE hardware constant One)
A2 = 0.3465735902632668
A0 = 0.41448987381943514
RND = 1.5 * 2.0**33      # adding this rounds an f32 to a multiple of 2^10


def _expf16_reference(in0, in1, s0, s1, imm2):
    f = np.float32
    T = np.asarray(in0, f)
    c0, c1, c2 = f(s0), f(s1), f(imm2)
    s1_ = (T + c0).astype(f)
    s2_ = (s1_ - c0).astype(f)
    s3_ = (T - s2_).astype(f)
    s6_ = (((s3_ * c1).astype(f) + f(1.0)).astype(f) * s3_).astype(f)
    return ((s6_ + c2).astype(f) + s2_).astype(f)


def _register_expf16():
    """Register the EXPF16_ANT custom DVE op at runtime (idempotent)."""
    import concourse.dve_ops as dve_ops
    from concourse.dve_spec import Spec, Src0, C0, C1, C2, One, lower
    from concourse.dve_uop import DveOpSpec

    for op in dve_ops.OPS:
        if op.name == "EXPF16_ANT":
            return op

    s1 = Src0 + C0          # C0 = 1.5*2^33
    s2 = s1 - C0            # q*2^10
    s3 = Src0 - s2          # v*2^10
    s4 = s3 * C1            # C1 = A2/2^10
    s5 = s4 + One           # pinned a1 = 1.0
    s6 = s5 * s3            # (A2*v + 1)*v*2^10
    s7 = s6 + C2            # C2 = A0*2^10
    body = s7 + s2          # (q + p(v))*2^10

    op = dve_ops.DveOp(
        "EXPF16_ANT",
        Spec(body=body, reference=_expf16_reference),
        subdim=False,
        uops_sha={},
    )
    dve_ops.OPS.append(op)
    dve_ops.CUSTOM_DVE_SPECS[op.name] = op.spec
    row = dve_ops._CUSTOM_DVE_ROW_BASE + len(dve_ops.OPS) - 1
    assert row < 0x20, "custom DVE opcode row overflow"
    dve_ops._SUB_OPCODE_FOR_NAME[op.name] = row
    for ver in ("v3", "v4"):
        uops = lower(op.spec, ver=ver)
        spec = DveOpSpec(name=op.name, opcode=row, uops=uops, rd1_en=False)
        op.uops_sha[ver] = spec.sha(ver)
    return op


def quad_engine(idx):
    return ASSIGN_PATTERN[idx % len(ASSIGN_PATTERN)]


def build_program(rows=ROWS, m_cols=M, repeats=1):
    """Build the per-core Bass program. rows/m_cols shrinkable for sim."""
    op = _register_expf16()
    rb = rows // P

    nc = bacc.Bacc()
    lta = nc.declare_dram_parameter("lta", [KA, rows], BF16, isOutput=False)
    rta = nc.declare_dram_parameter("rta", [KA, m_cols], BF16, isOutput=False)
    out = nc.declare_dram_parameter("out", [rows, m_cols], U16, isOutput=True)

    with tile.TileContext(nc) as tc:
        with (
            tc.tile_pool(name="singles", bufs=1) as singles,
            tc.tile_pool(name="psum_pool", bufs=2, space="PSUM") as psum_pool,
            tc.tile_pool(name="obuf_pool", bufs=4) as obuf_pool,
        ):
            # --- preamble ------------------------------------------------
            # Operands carry 2 copies at partitions 0/64 for the 2-way
            # tile_position row tiling.
            biasC = singles.tile([P, 1], F32)
            nc.vector.memset(biasC[:], -14.5 * LN2)
            LT = singles.tile([64 + KA, rows], BF16)
            RT = singles.tile([64 + KA, m_cols], BF16)
            for g in range(2):
                o = 64 * g
                nc.sync.dma_start(LT[o : o + KA, :], lta[:])
                nc.sync.dma_start(RT[o : o + KA, :], rta[:])

            # --- main loop ----------------------------------------------
            qidx = 0
            for _rep in range(repeats):
                for m in range(rb):
                    msl = slice(m * P, (m + 1) * P)
                    for sp in range(m_cols // STORE_COLS):
                        first = m == 0 and sp == 0
                        ob = obuf_pool.tile([P, STORE_COLS], U16, tag="ob",
                                            name="ob")
                        for s in range(STORE_COLS // QUAD):
                            q = sp * (STORE_COLS // QUAD) + s
                            ps = psum_pool.tile([P, QUAD], F32, tag="ps",
                                                name="ps")
                            for k in range(QUAD // FREE):
                                n = q * (QUAD // FREE) + k
                                # first span runs in row group 0 so it only
                                # waits for the first RT copy's DMA
                                o = 0 if first else 64 * (k % 2)
                                csl = slice(k * FREE, (k + 1) * FREE)
                                nsl = slice(n * FREE, (n + 1) * FREE)
                                nc.tensor.matmul(
                                    ps[:, csl],
                                    LT[o : o + KA, msl], RT[o : o + KA, nsl],
                                    start=True, stop=True,
                                    tile_position=(o, 0),
                                )
                            osl = slice(s * QUAD, (s + 1) * QUAD)
                            if quad_engine(qidx) == "A":
                                nc.scalar.activation(
                                    ob[:, osl].bitcast(F16), ps[:, :],
                                    AF.Exp, bias=biasC[:, 0:1],
                                    scale=LN2 / 2.0**10,
                                )
                            else:
                                nc.vector._custom_dve(
                                    op, out=ob[:, osl], in0=ps[:, :],
                                    s0=RND, s1=A2 / 2.0**10,
                                    imm2=A0 * 2.0**10,
                                )
                            qidx += 1
                        # the first span goes out in 2048-wide stores so the
                        # store stream starts before the preamble finishes
                        nst = (STORE_COLS // QUAD) if first else 1
                        w = STORE_COLS // nst
                        base = sp * STORE_COLS
                        for t in range(nst):
                            nc.sync.dma_start(
                                out[msl, base + t * w : base + (t + 1) * w],
                                ob[:, t * w : (t + 1) * w],
                            )

    nc.compile()
    return nc


_PROGRAM_CACHE = {}


def get_program(rows=ROWS, m_cols=M, repeats=1):
    key = (rows, m_cols, repeats, ASSIGN_PATTERN, STORE_COLS)
    if key not in _PROGRAM_CACHE:
        _PROGRAM_CACHE[key] = build_program(rows, m_cols, repeats)
    return _PROGRAM_CACHE[key]


def _bf16_3split(x):
    """Split f64 vector into 3 bf16 parts summing to ~27-bit accuracy."""
    a = x.astype(np.float32).astype(NP_BF16)
    r = x - a.astype(np.float64)
    b = r.astype(np.float32).astype(NP_BF16)
    r = r - b.astype(np.float64)
    c = r.astype(np.float32).astype(NP_BF16)
    return a, b, c


def make_in_maps(U, V, weights, sn):
    U = np.asarray(U, dtype=np.float32)
    V = np.asarray(V, dtype=np.float32)
    w = np.asarray(weights, dtype=np.float32).reshape(D)
    sn_f = np.float64(np.asarray(sn, dtype=np.float32))

    s = np.exp(-w.astype(np.float64))
    s2 = s * s

    # Right operand stack (replicated); row k of lta pairs with row k of
    # rta in the contraction:
    #   [0:D)        Ahi x Bhi     (hi*hi dot term)
    #   [D:D+3)      1   x col     (-0.5*ALPHA*||Vs_j||^2, 3-way split)
    #   [D+3:D+6)    row x 1       (ALPHA*(2sn-0.5||Us_i||^2)+14.5*2^10)
    #   [D+6:2D+6)   Ahi x Blo     (hi*lo cross term)
    #   [2D+6:3D+6)  Alo x Bhi     (lo*hi cross term)
    Vt = V.T.astype(np.float64)                     # (D, M)
    Bhi = Vt.astype(np.float32).astype(NP_BF16)
    Blo = (Vt - Bhi.astype(np.float64)).astype(np.float32).astype(NP_BF16)
    v2 = ((Vt * s[:, None]) ** 2).sum(axis=0)       # (M,)
    ca, cb, cc = _bf16_3split(-0.5 * ALPHA * v2)
    rta = np.empty((KA, M), dtype=NP_BF16)
    rta[0:D] = Bhi
    rta[D : D + 3] = np.stack([ca, cb, cc])
    rta[D + 3 : D + 6] = 1.0
    rta[D + 6 : 2 * D + 6] = Blo
    rta[2 * D + 6 : 3 * D + 6] = Bhi
    rta = np.ascontiguousarray(rta)

    in_maps = []
    for c in range(N_CORES):
        Uc = U[c * ROWS : (c + 1) * ROWS].astype(np.float64)    # (ROWS, D)
        Ap = (ALPHA * (Uc * s2)).T                              # (D, ROWS)
        Ahi = Ap.astype(np.float32).astype(NP_BF16)
        Alo = (Ap - Ahi.astype(np.float64)).astype(np.float32).astype(NP_BF16)
        u2 = ((Uc * s) ** 2).sum(axis=1)                        # (ROWS,)
        rowb = ALPHA * (2.0 * sn_f - 0.5 * u2) + EXP_BIAS
        ra, rb_, rc = _bf16_3split(rowb)
        lta = np.empty((KA, ROWS), dtype=NP_BF16)
        lta[0:D] = Ahi
        lta[D : D + 3] = 1.0
        lta[D + 3 : D + 6] = np.stack([ra, rb_, rc])
        lta[D + 6 : 2 * D + 6] = Ahi
        lta[2 * D + 6 : 3 * D + 6] = Alo
        in_maps.append({
            "lta": np.ascontiguousarray(lta),
            "rta": rta,
        })
    return in_maps


def decode_out(arr):
    """uint16 f16-bit-pattern array -> float32 values."""
    return arr.view(np.float16).astype(np.float32)


def kernel(U, V, weights, sn):
    from concourse.bass_utils import run_bass_kernel_spmd

    nc = get_program()
    in_maps = make_in_maps(U, V, weights, sn)
    res = run_bass_kernel_spmd(nc, in_maps, core_ids=list(range(N_CORES)))
    return np.concatenate([decode_out(r["out"]) for r in res.results], axis=0)
